# revision 17
# baseline (speedup 1.0000x reference)
"""GCN (3-layer, edge-weighted, mean-pool, classifier) on 8 TRN2 NeuronCores.

v2: gather via batched dma_gather (2 calls per block-group) instead of
per-128-edge indirect DMAs.  The baseline was bound by SWDGE instruction
issue on the Pool engine (~1us per indirect DMA x ~2700); dma_gather
moves the whole per-edge gather of a group into one instruction.

Layout: nodes are assigned to 8 cores x 49 blocks of 128 lanes.  Each
target node owns slot columns holding its in-edges (self-loop included
as a slot with P=+1e4 => sigmoid w=1).  Slots are split into "lo"
(source table row < 32768) and "hi" columns because dma_gather indices
are int16.  Nodes are labeled lo/hi by out-degree (top 32768 = lo,
which occupy table rows < 32768), then block assignment sorts nodes by
(in-deg, lo-count) so per-block slot maxima stay tight.

norm = dis[src]*w*dis[tgt] is factored: dis[src] folded into the table
(h~ = dis*h), w*dis[tgt] premultiplied into the slot weights.  Per
layer: AllGather the h~ table, per group of blocks 2 dma_gathers, per
block DVE multiply+reduce, PE transpose + [W;b] matmul (bias via ones
row), ACT relu*dis epilogue.  Pooling accumulates one-hot graph
matmuls in PSUM; partial logits (incl. bias via host-staged cnt/8 row)
are AllReduced as [64,10] and divided by host-staged counts.
"""
import sys

for p in ("/opt/trn_rl_repo", "/root/.axon_site/_ro/trn_rl_repo"):
    if p not in sys.path:
        sys.path.insert(0, p)

import numpy as np

import concourse.bacc as bacc
import concourse.bass as bass
import concourse.mybir as mybir
import concourse.tile as tile
from concourse import bass_utils
from concourse.masks import make_identity

N_NODES = 50000
N_EDGES = 800000
F = 64
N_CLASSES = 10
N_GRAPHS = 64
NC = 8
NPC = 6272                # node slots per core (49 blocks of 128)
NB = NPC // 128           # 49
SLOTS = NC * NPC          # 50176
LO_ROWS = 32768           # int16 index limit for dma_gather
GCAP = 32                 # max gather columns (128 descs each) per call

_cache = {}


def _host_prep(x, edge_index, batch, P_vec):
    """Slot layout + per-core input arrays (pure index manipulation)."""
    src = np.asarray(edge_index[0], np.int64)
    tgt = np.asarray(edge_index[1], np.int64)
    batch = np.asarray(batch, np.int64)
    P_vec = np.asarray(P_vec, np.float32)
    x = np.asarray(x, np.float32)

    # ---- phase A: lo/hi class label (top LO_ROWS by out-degree) ----
    outdeg = np.bincount(src, minlength=N_NODES)
    by_out = np.argsort(-outdeg, kind="stable")
    is_lo = np.zeros(N_NODES, bool)
    is_lo[by_out[:LO_ROWS]] = True

    # per-target lo/hi in-slot counts (in-edges + self)
    indeg_lo = np.bincount(tgt[is_lo[src]], minlength=N_NODES) + is_lo
    indeg_hi = np.bincount(tgt[~is_lo[src]], minlength=N_NODES) + (~is_lo)

    # ---- assign nodes to table rows (block-major deal per class) ----
    # table row r = core*NPC + pos; lo rows are r < LO_ROWS.  Order rows
    # block-major so sorted nodes land in tight blocks across cores.
    r = np.arange(SLOTS)
    core_r, pos_r = r // NPC, r % NPC
    blk_r = pos_r // 128
    order_rows = np.lexsort((core_r, blk_r))   # block-major, core minor
    lo_rows = order_rows[r[order_rows] < LO_ROWS]
    hi_rows = order_rows[r[order_rows] >= LO_ROWS]

    key = np.lexsort((-indeg_hi, -indeg_lo))   # (lo desc, hi desc)
    lo_nodes = key[is_lo[key]]
    hi_nodes = key[~is_lo[key]]

    row_of_node = np.empty(N_NODES, np.int64)
    row_of_node[lo_nodes] = lo_rows[:len(lo_nodes)]
    row_of_node[hi_nodes] = hi_rows[:len(hi_nodes)]
    pad_rows = np.concatenate(
        [lo_rows[len(lo_nodes):], hi_rows[len(hi_nodes):]])

    core_of = row_of_node // NPC
    pos_of = row_of_node % NPC

    # ---- per-block slot maxima (global across cores: SPMD-uniform) ----
    blk_of_node = pos_of // 128
    Klo = np.zeros(NB, np.int64)
    Khi = np.zeros(NB, np.int64)
    np.maximum.at(Klo, blk_of_node, indeg_lo)
    np.maximum.at(Khi, blk_of_node, indeg_hi)
    if len(pad_rows):
        # pads (always hi-class rows) need one unit-weight slot: deg>0
        np.maximum.at(Khi, (pad_rows % NPC) // 128, 1)
    cb_lo = np.concatenate([[0], np.cumsum(Klo)])
    cb_hi = np.concatenate([[0], np.cumsum(Khi)])
    C_lo, C_hi = int(cb_lo[-1]), int(cb_hi[-1])
    C = C_lo + C_hi

    # ---- slot rank of each edge within its target's lo/hi list ----
    # (the target's self slot takes rank 0 of the target's own class)
    srow = row_of_node[src]
    e_lo = srow < LO_ROWS
    o = np.argsort(tgt, kind="stable")
    st, sl = tgt[o], e_lo[o]
    grp_first = np.r_[True, st[1:] != st[:-1]]
    gstart = np.flatnonzero(grp_first)
    glen = np.diff(np.r_[gstart, len(st)])
    gidx = np.repeat(np.arange(len(gstart)), glen)
    within = np.arange(len(st)) - gstart[gidx]
    lo_excl = np.cumsum(sl) - sl              # lo edges before, global
    lo_before = lo_excl - lo_excl[gstart][gidx]
    hi_before = within - lo_before
    rank = np.where(sl, lo_before, hi_before) + (sl == is_lo[st])
    kslot = np.empty(len(st), np.int64)
    kslot[o] = rank

    tcore = core_of[tgt]
    tlane = pos_of[tgt] % 128
    tblk = blk_of_node[tgt]

    idx_arr = np.zeros((NC, 128, C), np.int32)
    P_arr = np.full((NC, 128, C), -1e4, np.float32)

    ccol = np.where(e_lo, cb_lo[tblk] + kslot, C_lo + cb_hi[tblk] + kslot)
    idx_arr[tcore, tlane, ccol] = np.where(e_lo, srow, srow - LO_ROWS)
    P_arr[tcore, tlane, ccol] = P_vec

    # self slots (rank 0 of own class) for real nodes
    selfc = np.where(is_lo, cb_lo[blk_of_node], C_lo + cb_hi[blk_of_node])
    idx_arr[core_of, pos_of % 128, selfc] = np.where(
        is_lo, row_of_node, row_of_node - LO_ROWS)
    P_arr[core_of, pos_of % 128, selfc] = 1e4

    # pad rows (all hi-class): one unit-weight slot so deg>0 (dis finite)
    if len(pad_rows):
        pc, pp = pad_rows // NPC, pad_rows % NPC
        padc = C_lo + cb_hi[pp // 128]
        idx_arr[pc, pp % 128, padc] = 0
        P_arr[pc, pp % 128, padc] = 1e4

    # wrapped int16 index arrays: gather position i=(col-local)*128+lane,
    # index at [i%16, i//16] within the call's column span => globally
    # lane p of column c sits at [p%16 (+16g), c*8 + p//16]
    def wrap16(arr):
        ncx, _, cc = arr.shape
        if cc == 0:
            return np.zeros((ncx, 128, 8), np.int16)
        w = arr.transpose(0, 2, 1).reshape(ncx, cc, 8, 16)
        w = w.transpose(0, 3, 1, 2).reshape(ncx, 16, cc * 8)
        out = np.zeros((ncx, 128, cc * 8), np.int16)
        for g in range(8):
            out[:, g * 16:(g + 1) * 16, :] = w
        return out

    ilo16 = wrap16(idx_arr[:, :, :C_lo].astype(np.int16))
    ihi16 = wrap16(idx_arr[:, :, C_lo:].astype(np.int16))

    gid_arr = np.full((NC, 128, NB), float(N_GRAPHS), np.float32)
    gid_arr[core_of, pos_of % 128, pos_of // 128] = batch.astype(np.float32)

    x_slots = np.zeros((NC, NPC, F), np.float32)
    x_slots[core_of, pos_of] = x

    cnt = np.bincount(batch, minlength=N_GRAPHS).astype(np.float32)
    cnt = np.maximum(cnt, 1.0)
    cntrow = (cnt / NC)[None, :]                     # [1, 64]
    cnt64 = cnt[:, None]                             # [64, 1]

    return dict(
        Klo=[int(k) for k in Klo], Khi=[int(k) for k in Khi],
        C_lo=C_lo, C_hi=C_hi,
        idx=idx_arr, P=P_arr, ilo16=ilo16, ihi16=ihi16,
        gid=gid_arr, x_slots=x_slots, cntrow=cntrow, cnt64=cnt64,
    )


def _groups(Klo, Khi, target_span):
    """Contiguous block ranges with ~target_span total columns each."""
    out = []
    b0 = 0
    acc = 0
    for b in range(NB):
        acc += Klo[b] + Khi[b]
        if acc >= target_span or b == NB - 1:
            out.append((b0, b + 1))
            b0 = b + 1
            acc = 0
    return out


def _build(Klo, Khi, C_lo, C_hi):
    f32 = mybir.dt.float32
    i16 = mybir.dt.int16
    C = C_lo + C_hi
    cb_lo = np.concatenate([[0], np.cumsum(Klo)]).astype(int)
    cb_hi = np.concatenate([[0], np.cumsum(Khi)]).astype(int)
    groups = _groups(Klo, Khi, (C + 11) // 12)
    maxspan = max(
        (cb_lo[b1] - cb_lo[b0]) + (cb_hi[b1] - cb_hi[b0]) for b0, b1 in groups)

    nc = bacc.Bacc("TRN2", target_bir_lowering=False, debug=False,
                   num_devices=NC)

    x_own = nc.dram_tensor("x_own", [NPC, F], f32, kind="ExternalInput")
    p_in = nc.dram_tensor("pv", [128, C], f32, kind="ExternalInput")
    ilo_in = nc.dram_tensor("ilo", [128, 8 * max(C_lo, 1)], i16,
                            kind="ExternalInput")
    ihi_in = nc.dram_tensor("ihi", [128, 8 * max(C_hi, 1)], i16,
                            kind="ExternalInput")
    gid_in = nc.dram_tensor("gid", [128, NB], f32, kind="ExternalInput")
    iota_in = nc.dram_tensor("iota64", [128, F], f32, kind="ExternalInput")
    waug_in = [nc.dram_tensor(f"waug{l}", [F + 1, F], f32,
                              kind="ExternalInput") for l in range(3)]
    wl_in = nc.dram_tensor("wlaug", [F + 1, N_CLASSES], f32,
                           kind="ExternalInput")
    cntrow_in = nc.dram_tensor("cntrow", [1, N_GRAPHS], f32,
                               kind="ExternalInput")
    cnt64_in = nc.dram_tensor("cnt64", [N_GRAPHS, 1], f32,
                              kind="ExternalInput")
    out_d = nc.dram_tensor("out", [N_GRAPHS, N_CLASSES], f32,
                           kind="ExternalOutput")

    with tile.TileContext(nc) as tc:
        with tc.tile_pool(name="const", bufs=1) as cp, \
             tc.tile_pool(name="meta", bufs=1) as mp, \
             tc.tile_pool(name="work", bufs=3) as wp, \
             tc.tile_pool(name="msgs", bufs=2) as gp, \
             tc.tile_pool(name="psA", bufs=2, space="PSUM") as psA, \
             tc.tile_pool(name="psB", bufs=2, space="PSUM") as psB, \
             tc.tile_pool(name="psP", bufs=1, space="PSUM") as psP, \
             tc.tile_pool(name="dram", bufs=1, space="DRAM") as dp:

            ident = cp.tile([128, 128], f32)
            make_identity(nc, ident[:])
            iota_sb = cp.tile([128, F], f32)
            nc.sync.dma_start(out=iota_sb[:], in_=iota_in[:, :])
            waug_sb = []
            for l in range(3):
                t = cp.tile([F + 1, F], f32, tag=f"waug{l}")
                nc.sync.dma_start(out=t[:], in_=waug_in[l][:, :])
                waug_sb.append(t)
            wl_sb = cp.tile([F + 1, N_CLASSES], f32)
            nc.sync.dma_start(out=wl_sb[:], in_=wl_in[:, :])
            cnt64_sb = cp.tile([N_GRAPHS, 1], f32)
            nc.sync.dma_start(out=cnt64_sb[:], in_=cnt64_in[:, :])

            ilo_sb = mp.tile([128, 8 * max(C_lo, 1)], i16)
            nc.sync.dma_start(out=ilo_sb[:], in_=ilo_in[:, :])
            ihi_sb = mp.tile([128, 8 * max(C_hi, 1)], i16)
            nc.sync.dma_start(out=ihi_sb[:], in_=ihi_in[:, :])
            w_sb = mp.tile([128, C], f32)
            gid_sb = mp.tile([128, NB], f32)
            nc.sync.dma_start(out=gid_sb[:], in_=gid_in[:, :])
            dis_sb = mp.tile([128, NB], f32)
            gmat = mp.tile([128, NB * N_GRAPHS], f32)
            hout = mp.tile([128, NB * F], f32)

            # ---- prepass ----
            p_sb = wp.tile([128, C], f32, tag="ptmp")
            nc.sync.dma_start(out=p_sb[:], in_=p_in[:, :])
            nc.scalar.activation(out=w_sb[:], in_=p_sb[:],
                                 func=mybir.ActivationFunctionType.Sigmoid)
            deg_sb = wp.tile([128, NB], f32, tag="deg")
            deg2_sb = wp.tile([128, NB], f32, tag="deg2")
            nc.vector.memset(deg_sb[:], 0.0)
            nc.vector.memset(deg2_sb[:], 0.0)
            for b in range(NB):
                if Klo[b]:
                    nc.vector.tensor_reduce(
                        out=deg_sb[:, b:b + 1],
                        in_=w_sb[:, cb_lo[b]:cb_lo[b] + Klo[b]],
                        axis=mybir.AxisListType.X, op=mybir.AluOpType.add)
                if Khi[b]:
                    nc.vector.tensor_reduce(
                        out=deg2_sb[:, b:b + 1],
                        in_=w_sb[:, C_lo + cb_hi[b]:C_lo + cb_hi[b] + Khi[b]],
                        axis=mybir.AxisListType.X, op=mybir.AluOpType.add)
            nc.vector.tensor_tensor(out=deg_sb[:], in0=deg_sb[:],
                                    in1=deg2_sb[:], op=mybir.AluOpType.add)
            nc.scalar.activation(out=deg_sb[:], in_=deg_sb[:],
                                 func=mybir.ActivationFunctionType.Sqrt)
            nc.vector.reciprocal(out=dis_sb[:], in_=deg_sb[:])
            # premultiply slot weights by dis[tgt]
            for b in range(NB):
                if Klo[b]:
                    s = slice(cb_lo[b], cb_lo[b] + Klo[b])
                    nc.vector.tensor_tensor(
                        out=w_sb[:, s], in0=w_sb[:, s],
                        in1=dis_sb[:, b:b + 1].to_broadcast([128, Klo[b]]),
                        op=mybir.AluOpType.mult)
                if Khi[b]:
                    s = slice(C_lo + cb_hi[b], C_lo + cb_hi[b] + Khi[b])
                    nc.vector.tensor_tensor(
                        out=w_sb[:, s], in0=w_sb[:, s],
                        in1=dis_sb[:, b:b + 1].to_broadcast([128, Khi[b]]),
                        op=mybir.AluOpType.mult)
            # one-hot graph matrices (constant across layers)
            for b in range(NB):
                nc.vector.tensor_tensor(
                    out=gmat[:, b * N_GRAPHS:(b + 1) * N_GRAPHS],
                    in0=gid_sb[:, b:b + 1].to_broadcast([128, N_GRAPHS]),
                    in1=iota_sb[:],
                    op=mybir.AluOpType.is_equal)
            # x~ = dis * x
            xs = wp.tile([128, NB * F], f32, tag="xload")
            nc.sync.dma_start(
                out=xs[:].rearrange("p (b f) -> p b f", f=F),
                in_=x_own[:, :].rearrange("(b p) f -> p b f", p=128))
            nc.vector.tensor_tensor(
                out=hout[:].rearrange("p (b f) -> p b f", f=F),
                in0=xs[:].rearrange("p (b f) -> p b f", f=F),
                in1=dis_sb[:].to_broadcast([128, NB, F]),
                op=mybir.AluOpType.mult)

            agins = [dp.tile([NPC, F], f32, name=f"agin{l}", tag=f"agin{l}")
                     for l in range(3)]
            agouts = [dp.tile([SLOTS, F], f32, addr_space="Shared",
                              name=f"agout{l}", tag=f"ag{l}")
                      for l in range(3)]
            pool_ps = psP.tile([N_GRAPHS, F], f32)

            nc.sync.dma_start(
                out=agins[0][:].rearrange("(b p) f -> p b f", p=128),
                in_=hout[:].rearrange("p (b f) -> p b f", f=F))

            for l in range(3):
                nc.gpsimd.collective_compute(
                    "AllGather", mybir.AluOpType.bypass,
                    ins=[agins[l][:]], outs=[agouts[l][:]],
                    replica_groups=[list(range(NC))])
                src = agouts[l]

                for b0, b1 in groups:
                    clo0, clo1 = cb_lo[b0], cb_lo[b1]
                    chi0, chi1 = cb_hi[b0], cb_hi[b1]
                    slo, shi = clo1 - clo0, chi1 - chi0
                    msg = gp.tile([128, maxspan * F], f32, tag="msg")
                    # descriptor-ring limit: <= GCAP descriptors per gather
                    for q0 in range(0, slo, GCAP):
                        qn = min(GCAP, slo - q0)
                        nc.gpsimd.dma_gather(
                            msg[:, q0 * F:(q0 + qn) * F].rearrange(
                                "p (c f) -> p c f", f=F),
                            src[:, :],
                            ilo_sb[:, 8 * (clo0 + q0):8 * (clo0 + q0 + qn)],
                            128 * qn, 128 * qn, F, single_packet=False)
                    for q0 in range(0, shi, GCAP):
                        qn = min(GCAP, shi - q0)
                        nc.gpsimd.dma_gather(
                            msg[:, (slo + q0) * F:(slo + q0 + qn) * F]
                                .rearrange("p (c f) -> p c f", f=F),
                            src[LO_ROWS:, :],
                            ihi_sb[:, 8 * (chi0 + q0):8 * (chi0 + q0 + qn)],
                            128 * qn, 128 * qn, F, single_packet=False)
                    for b in range(b0, b1):
                        klo, khi = Klo[b], Khi[b]
                        lob = cb_lo[b] - clo0
                        hib = slo + (cb_hi[b] - chi0)
                        agg = wp.tile([128, F], f32, tag="agg")
                        if klo:
                            ms = msg[:, lob * F:(lob + klo) * F]
                            nc.vector.tensor_tensor(
                                out=ms.rearrange("p (k f) -> p k f", f=F),
                                in0=ms.rearrange("p (k f) -> p k f", f=F),
                                in1=w_sb[:, cb_lo[b]:cb_lo[b] + klo]
                                    .to_broadcast([128, klo, F]),
                                op=mybir.AluOpType.mult)
                            nc.vector.tensor_reduce(
                                out=agg[:],
                                in_=ms.rearrange("p (k f) -> p f k", f=F),
                                axis=mybir.AxisListType.X,
                                op=mybir.AluOpType.add)
                        if khi:
                            ms = msg[:, hib * F:(hib + khi) * F]
                            nc.vector.tensor_tensor(
                                out=ms.rearrange("p (k f) -> p k f", f=F),
                                in0=ms.rearrange("p (k f) -> p k f", f=F),
                                in1=w_sb[:, C_lo + cb_hi[b]:
                                         C_lo + cb_hi[b] + khi]
                                    .to_broadcast([128, khi, F]),
                                op=mybir.AluOpType.mult)
                            if klo:
                                agg2 = wp.tile([128, F], f32, tag="agg2")
                                nc.vector.tensor_reduce(
                                    out=agg2[:],
                                    in_=ms.rearrange("p (k f) -> p f k", f=F),
                                    axis=mybir.AxisListType.X,
                                    op=mybir.AluOpType.add)
                                nc.vector.tensor_tensor(
                                    out=agg[:], in0=agg[:], in1=agg2[:],
                                    op=mybir.AluOpType.add)
                            else:
                                nc.vector.tensor_reduce(
                                    out=agg[:],
                                    in_=ms.rearrange("p (k f) -> p f k", f=F),
                                    axis=mybir.AxisListType.X,
                                    op=mybir.AluOpType.add)
                        tp = psA.tile([F, 128], f32, tag="tp")
                        nc.tensor.transpose(out=tp[:], in_=agg[:],
                                            identity=ident[:])
                        aug = wp.tile([F + 1, 128], f32, tag="aug")
                        nc.vector.memset(aug[F:F + 1, :], 1.0)
                        nc.vector.tensor_copy(out=aug[:F, :], in_=tp[:])
                        gps = psB.tile([128, F], f32, tag="g")
                        nc.tensor.matmul(out=gps[:], lhsT=aug[:],
                                         rhs=waug_sb[l][:],
                                         start=True, stop=True)
                        if l < 2:
                            nc.scalar.activation(
                                out=hout[:, b * F:(b + 1) * F], in_=gps[:],
                                func=mybir.ActivationFunctionType.Relu,
                                scale=dis_sb[:, b:b + 1])
                        else:
                            h3 = wp.tile([128, F], f32, tag="h3")
                            nc.vector.tensor_copy(out=h3[:], in_=gps[:])
                            nc.tensor.matmul(
                                out=pool_ps[:],
                                lhsT=gmat[:, b * N_GRAPHS:(b + 1) * N_GRAPHS],
                                rhs=h3[:],
                                start=(b == 0), stop=(b == NB - 1))
                    if l < 2:
                        nc.sync.dma_start(
                            out=agins[l + 1][:].rearrange(
                                "(bb p) f -> p bb f", p=128)[:, b0:b1, :],
                            in_=hout[:, b0 * F:b1 * F].rearrange(
                                "p (b f) -> p b f", f=F))

            # ---- pooling epilogue ----
            poolin = wp.tile([N_GRAPHS, F], f32, tag="poolin")
            nc.vector.tensor_copy(out=poolin[:], in_=pool_ps[:])
            tp2 = psA.tile([F, N_GRAPHS], f32, tag="tp")
            nc.tensor.transpose(out=tp2[:], in_=poolin[:],
                                identity=ident[:N_GRAPHS, :N_GRAPHS])
            tpool = wp.tile([F + 1, N_GRAPHS], f32, tag="tpool")
            nc.vector.tensor_copy(out=tpool[:F, :], in_=tp2[:])
            nc.sync.dma_start(out=tpool[F:F + 1, :], in_=cntrow_in[:, :])
            plog = psB.tile([N_GRAPHS, N_CLASSES], f32, tag="g")
            nc.tensor.matmul(out=plog[:], lhsT=tpool[:], rhs=wl_sb[:],
                             start=True, stop=True)
            plog_sb = wp.tile([N_GRAPHS, N_CLASSES], f32, tag="plogsb")
            nc.vector.tensor_copy(out=plog_sb[:], in_=plog[:])
            arin = dp.tile([N_GRAPHS, N_CLASSES], f32, tag="arin")
            arout = dp.tile([N_GRAPHS, N_CLASSES], f32, addr_space="Shared",
                            tag="arout")
            nc.sync.dma_start(out=arin[:], in_=plog_sb[:])
            nc.gpsimd.collective_compute(
                "AllReduce", mybir.AluOpType.add,
                ins=[arin[:]], outs=[arout[:]],
                replica_groups=[list(range(NC))])
            ar_sb = wp.tile([N_GRAPHS, N_CLASSES], f32, tag="arsb")
            nc.sync.dma_start(out=ar_sb[:], in_=arout[:])
            rec = wp.tile([N_GRAPHS, 1], f32, tag="rec")
            nc.vector.reciprocal(out=rec[:], in_=cnt64_sb[:])
            out_sb = wp.tile([N_GRAPHS, N_CLASSES], f32, tag="outsb")
            nc.vector.tensor_scalar_mul(out_sb[:], ar_sb[:], rec[:])
            nc.sync.dma_start(out=out_d[:, :], in_=out_sb[:])

    nc.compile()
    return nc


def _run(inputs, trace=False):
    x = inputs["x"]
    prep = _host_prep(x, inputs["edge_index"], inputs["batch"],
                      inputs["P_vec"])
    key = ("v2", prep["C_lo"], prep["C_hi"],
           tuple(prep["Klo"]), tuple(prep["Khi"]))
    if key not in _cache:
        _cache.clear()
        _cache[key] = _build(prep["Klo"], prep["Khi"],
                             prep["C_lo"], prep["C_hi"])
    nc = _cache[key]

    waugs = []
    for (W, b) in [(inputs["W1"], inputs["b1"]), (inputs["W2"], inputs["b2"]),
                   (inputs["W3"], inputs["b3"])]:
        waugs.append(np.concatenate(
            [np.asarray(W, np.float32),
             np.asarray(b, np.float32)[None, :]], axis=0))
    wlaug = np.concatenate(
        [np.asarray(inputs["Wl"], np.float32),
         np.asarray(inputs["bl"], np.float32)[None, :]], axis=0)
    iota64 = np.tile(np.arange(F, dtype=np.float32)[None, :], (128, 1))

    in_maps = []
    for c in range(NC):
        in_maps.append({
            "x_own": prep["x_slots"][c],
            "pv": prep["P"][c],
            "ilo": prep["ilo16"][c], "ihi": prep["ihi16"][c],
            "gid": prep["gid"][c], "iota64": iota64,
            "waug0": waugs[0], "waug1": waugs[1], "waug2": waugs[2],
            "wlaug": wlaug,
            "cntrow": prep["cntrow"], "cnt64": prep["cnt64"],
        })

    res = bass_utils.run_bass_kernel_spmd(
        nc, in_maps, core_ids=list(range(NC)), trace=trace)
    return res.results[0]["out"].astype(np.float32), res


def kernel(**inputs) -> np.ndarray:
    out, _ = _run(inputs, trace=False)
    return out


# revision 18
# speedup vs baseline: 1.0960x; 1.0960x over previous
"""GCN (3-layer, edge-weighted, mean-pool, classifier) on 8 TRN2 NeuronCores.

Strategy (sharding_hint: shard nodes + incident edges across cores):
- Nodes are assigned to 8 cores round-robin by in-degree rank, so each
  core gets ~6250 nodes in 49 blocks of 128 with near-uniform in-degree
  per block.  Each target node owns K slots (its in-edges incl. the
  self-loop, padded to the block max K_b).
- norm = dis[src] * w_e * dis[tgt] is factored: dis[src] is folded into
  the gathered table (h~ = dis * h), w_e is applied per-slot on DVE,
  dis[tgt] is applied per-partition after aggregation.
- Per layer: every core gathers h~[src] rows from a replicated DRAM
  table (one indirect DMA per 128-edge chunk), multiplies by w, reduces
  slots on DVE, then per 128-node block: transpose (PE), augmented
  matmul with [W; b] (bias via ones row), relu*dis on ACT.  Layers
  exchange h~ via AllGather.  Pooling = one-hot graph matmul into an
  accumulating PSUM bank, AllReduce, then a tiny classifier matmul.
"""
import sys

for p in ("/opt/trn_rl_repo", "/root/.axon_site/_ro/trn_rl_repo"):
    if p not in sys.path:
        sys.path.insert(0, p)

import numpy as np

import concourse.bacc as bacc
import concourse.bass as bass
import concourse.mybir as mybir
import concourse.tile as tile
from concourse import bass_utils
from concourse.masks import make_identity

N_NODES = 50000
N_EDGES = 800000
F = 64
N_CLASSES = 10
N_GRAPHS = 64
NC = 8
NPC = 6272                # node slots per core (49 blocks of 128)
NB = NPC // 128           # 49
SLOTS = NC * NPC          # 50176

_cache = {}


def _host_prep(x, edge_index, batch, P_vec):
    """Slot layout + per-core input arrays (pure index manipulation)."""
    row = np.asarray(edge_index[0], np.int64)
    col = np.asarray(edge_index[1], np.int64)
    batch = np.asarray(batch, np.int64)
    P_vec = np.asarray(P_vec, np.float32)
    x = np.asarray(x, np.float32)

    deg = np.bincount(col, minlength=N_NODES)      # self loop handled on-chip
    order = np.argsort(-deg, kind="stable")        # nodes by degree desc
    r_of_node = np.empty(N_NODES, np.int64)
    r_of_node[order] = np.arange(N_NODES)
    core_of = r_of_node % NC
    pos_of = r_of_node // NC                       # < 6250
    slotrow_of = core_of * NPC + pos_of

    # graph edges only (self loops are the identity contribution, added
    # from the core's own SBUF h~ block); w = sigmoid(P)
    esrc = row
    etgt = col
    eP = P_vec

    # slot rank k of each edge within its target
    o = np.argsort(etgt, kind="stable")
    sk = etgt[o]
    grp_first = np.r_[True, sk[1:] != sk[:-1]]
    gstart = np.flatnonzero(grp_first)
    glen = np.diff(np.r_[gstart, len(sk)])
    kslot_sorted = np.arange(len(sk)) - np.repeat(gstart, glen)
    kslot = np.empty(len(sk), np.int64)
    kslot[o] = kslot_sorted

    # per-block chunk count (global across cores -> SPMD-uniform program)
    block_of_node = pos_of // 128
    Kb = np.zeros(NB, np.int64)
    np.maximum.at(Kb, block_of_node, deg)
    Kb = np.maximum(Kb, 1)
    cbase = np.r_[0, np.cumsum(Kb)][:-1]
    C = int(Kb.sum())

    tcore = core_of[etgt]
    tlane = pos_of[etgt] % 128
    ccol = cbase[block_of_node[etgt]] + kslot

    idx_arr = np.zeros((NC, 128, C), np.int32)
    P_arr = np.full((NC, 128, C), -1e4, np.float32)  # pad: sigmoid -> ~0
    idx_arr[tcore, tlane, ccol] = slotrow_of[esrc]
    P_arr[tcore, tlane, ccol] = eP

    gid_arr = np.full((NC, 128, NB), float(N_GRAPHS), np.float32)
    gid_arr[core_of, pos_of % 128, pos_of // 128] = batch.astype(np.float32)

    x_slots = np.zeros((NC, NPC, F), np.float32)
    x_slots[core_of, pos_of] = x

    return dict(
        Kb=[int(k) for k in Kb], cbase=[int(c) for c in cbase], C=C,
        idx=idx_arr, P=P_arr, gid=gid_arr, x_slots=x_slots,
    )


def _build(Kb, cbase, C):
    f32 = mybir.dt.float32
    nc = bacc.Bacc("TRN2", target_bir_lowering=False, debug=False, num_devices=NC)

    x_own = nc.dram_tensor("x_own", [NPC, F], f32, kind="ExternalInput")
    idx_in = nc.dram_tensor("idx", [128, C], mybir.dt.int32, kind="ExternalInput")
    p_in = nc.dram_tensor("pv", [128, C], f32, kind="ExternalInput")
    gid_in = nc.dram_tensor("gid", [128, NB], f32, kind="ExternalInput")
    iota_in = nc.dram_tensor("iota64", [128, F], f32, kind="ExternalInput")
    waug_in = [nc.dram_tensor(f"waug{l}", [F + 1, F], f32, kind="ExternalInput")
               for l in range(3)]
    wl_in = nc.dram_tensor("wlaug", [F + 1, N_CLASSES], f32, kind="ExternalInput")
    out_d = nc.dram_tensor("out", [N_GRAPHS, N_CLASSES], f32, kind="ExternalOutput")

    with tile.TileContext(nc) as tc:
        with tc.tile_pool(name="const", bufs=1) as cp, \
             tc.tile_pool(name="meta", bufs=1) as mp, \
             tc.tile_pool(name="work", bufs=3) as wp, \
             tc.tile_pool(name="msgs", bufs=4) as gp, \
             tc.tile_pool(name="psA", bufs=2, space="PSUM") as psA, \
             tc.tile_pool(name="psB", bufs=2, space="PSUM") as psB, \
             tc.tile_pool(name="psP", bufs=1, space="PSUM") as psP, \
             tc.tile_pool(name="dram", bufs=1, space="DRAM") as dp:

            ident = cp.tile([128, 128], f32)
            make_identity(nc, ident[:])
            iota_sb = cp.tile([128, F], f32)
            nc.sync.dma_start(out=iota_sb[:], in_=iota_in[:, :])
            waug_sb = []
            for l in range(3):
                t = cp.tile([F + 1, F], f32, tag=f"waug{l}")
                nc.sync.dma_start(out=t[:], in_=waug_in[l][:, :])
                waug_sb.append(t)
            wl_sb = cp.tile([F + 1, N_CLASSES], f32)
            nc.sync.dma_start(out=wl_sb[:], in_=wl_in[:, :])

            idx_sb = mp.tile([128, C], mybir.dt.int32)
            nc.sync.dma_start(out=idx_sb[:], in_=idx_in[:, :])
            w_sb = mp.tile([128, C], f32)
            gid_sb = mp.tile([128, NB], f32)
            nc.sync.dma_start(out=gid_sb[:], in_=gid_in[:, :])
            dis_sb = mp.tile([128, NB], f32)

            # ---- prepass: w = sigmoid(P); dis = 1/sqrt(deg_w + 1); x~ ----
            p_sb = wp.tile([128, C], f32, tag="ptmp")
            nc.sync.dma_start(out=p_sb[:], in_=p_in[:, :])
            nc.scalar.activation(out=w_sb[:], in_=p_sb[:],
                                 func=mybir.ActivationFunctionType.Sigmoid)
            deg_sb = wp.tile([128, NB], f32, tag="deg")
            for b in range(NB):
                nc.vector.tensor_reduce(
                    out=deg_sb[:, b:b + 1],
                    in_=w_sb[:, cbase[b]:cbase[b] + Kb[b]],
                    axis=mybir.AxisListType.X, op=mybir.AluOpType.add)
            # + 1.0 for the self loop (weight exactly 1), sqrt on ACT
            nc.scalar.activation(out=deg_sb[:], in_=deg_sb[:],
                                 func=mybir.ActivationFunctionType.Sqrt,
                                 bias=1.0)
            nc.vector.reciprocal(out=dis_sb[:], in_=deg_sb[:])

            hout = mp.tile([128, NB * F], f32)  # per-core h~ blocks
            xs = wp.tile([128, NB * F], f32, tag="xload")
            nc.sync.dma_start(
                out=xs[:].rearrange("p (b f) -> p b f", f=F),
                in_=x_own[:, :].rearrange("(b p) f -> p b f", p=128))
            nc.vector.tensor_tensor(
                out=hout[:].rearrange("p (b f) -> p b f", f=F),
                in0=xs[:].rearrange("p (b f) -> p b f", f=F),
                in1=dis_sb[:].to_broadcast([128, NB, F]),
                op=mybir.AluOpType.mult)

            agins = [dp.tile([NPC, F], f32, name=f"agin{l}", tag=f"agin{l}")
                     for l in range(3)]
            agouts = [dp.tile([SLOTS, F], f32, addr_space="Shared",
                              name=f"agout{l}", tag=f"ag{l}") for l in range(3)]
            pool_ps = psP.tile([N_GRAPHS, F + 1], f32)

            nc.sync.dma_start(
                out=agins[0][:].rearrange("(b p) f -> p b f", p=128),
                in_=hout[:].rearrange("p (b f) -> p b f", f=F))
            for l in range(3):
                # layer 1/2 bounce buffers were already filled per-block by
                # the previous layer's epilogue DMAs
                nc.gpsimd.collective_compute(
                    "AllGather", mybir.AluOpType.bypass,
                    ins=[agins[l][:]], outs=[agouts[l][:]],
                    replica_groups=[list(range(NC))])
                src = agouts[l]

                for b in range(NB):
                    K = Kb[b]
                    msg = gp.tile([128, K * F], f32, tag="msg")
                    for k in range(K):
                        c = cbase[b] + k
                        nc.gpsimd.indirect_dma_start(
                            out=msg[:, k * F:(k + 1) * F],
                            out_offset=None,
                            in_=src[:],
                            in_offset=bass.IndirectOffsetOnAxis(
                                ap=idx_sb[:, c:c + 1], axis=0))
                    nc.vector.tensor_tensor(
                        out=msg[:].rearrange("p (k f) -> p k f", f=F),
                        in0=msg[:].rearrange("p (k f) -> p k f", f=F),
                        in1=w_sb[:, cbase[b]:cbase[b] + K].to_broadcast([128, K, F]),
                        op=mybir.AluOpType.mult)
                    agg = wp.tile([128, F], f32, tag="agg")
                    nc.vector.tensor_reduce(
                        out=agg[:],
                        in_=msg[:].rearrange("p (k f) -> p f k", f=F),
                        axis=mybir.AxisListType.X, op=mybir.AluOpType.add)
                    # self-loop: w=1 contribution is the core's own h~ block
                    nc.vector.tensor_tensor(
                        out=agg[:], in0=agg[:],
                        in1=hout[:, b * F:(b + 1) * F],
                        op=mybir.AluOpType.add)
                    nc.vector.tensor_scalar_mul(agg[:], agg[:], dis_sb[:, b:b + 1])
                    tp = psA.tile([F, 128], f32, tag="tp")
                    nc.tensor.transpose(out=tp[:], in_=agg[:], identity=ident[:])
                    aug = wp.tile([F + 1, 128], f32, tag="aug")
                    nc.vector.memset(aug[F:F + 1, :], 1.0)
                    nc.vector.tensor_copy(out=aug[:F, :], in_=tp[:])
                    gps = psB.tile([128, F], f32, tag="g")
                    nc.tensor.matmul(out=gps[:], lhsT=aug[:], rhs=waug_sb[l][:],
                                     start=True, stop=True)
                    if l < 2:
                        nc.scalar.activation(
                            out=hout[:, b * F:(b + 1) * F], in_=gps[:],
                            func=mybir.ActivationFunctionType.Relu,
                            scale=dis_sb[:, b:b + 1])
                        # ship this block to the next layer's AG bounce now,
                        # off the layer-boundary critical path
                        nc.sync.dma_start(
                            out=agins[l + 1][:].rearrange(
                                "(bb p) f -> p bb f", p=128)[:, b:b + 1, :],
                            in_=hout[:, b * F:(b + 1) * F])
                    else:
                        h3 = wp.tile([128, F + 1], f32, tag="h3")
                        nc.vector.memset(h3[:, F:F + 1], 1.0)
                        nc.vector.tensor_copy(out=h3[:, :F], in_=gps[:])
                        gmat = wp.tile([128, N_GRAPHS], f32, tag="gmat")
                        nc.vector.tensor_tensor(
                            out=gmat[:],
                            in0=gid_sb[:, b:b + 1].to_broadcast([128, N_GRAPHS]),
                            in1=iota_sb[:],
                            op=mybir.AluOpType.is_equal)
                        nc.tensor.matmul(out=pool_ps[:], lhsT=gmat[:], rhs=h3[:],
                                         start=(b == 0), stop=(b == NB - 1))

            # ---- pooling epilogue ----
            poolin = wp.tile([N_GRAPHS, F + 1], f32, tag="poolin")
            nc.vector.tensor_copy(out=poolin[:], in_=pool_ps[:])
            arin = dp.tile([N_GRAPHS, F + 1], f32, tag="arin")
            arout = dp.tile([N_GRAPHS, F + 1], f32, addr_space="Shared", tag="arout")
            nc.sync.dma_start(out=arin[:], in_=poolin[:])
            nc.gpsimd.collective_compute(
                "AllReduce", mybir.AluOpType.add,
                ins=[arin[:]], outs=[arout[:]],
                replica_groups=[list(range(NC))])
            ar_sb = wp.tile([N_GRAPHS, F + 1], f32, tag="arsb")
            nc.sync.dma_start(out=ar_sb[:], in_=arout[:])
            cnt = wp.tile([N_GRAPHS, 1], f32, tag="cnt")
            nc.vector.tensor_scalar_max(cnt[:], ar_sb[:, F:F + 1], 1.0)
            rec = wp.tile([N_GRAPHS, 1], f32, tag="rec")
            nc.vector.reciprocal(out=rec[:], in_=cnt[:])
            pooled = wp.tile([N_GRAPHS, F], f32, tag="pooled")
            nc.vector.tensor_scalar_mul(pooled[:], ar_sb[:, :F], rec[:])
            tp2 = psA.tile([F, N_GRAPHS], f32, tag="tp")
            nc.tensor.transpose(out=tp2[:], in_=pooled[:],
                                identity=ident[:N_GRAPHS, :N_GRAPHS])
            aug2 = wp.tile([F + 1, N_GRAPHS], f32, tag="aug2")
            nc.vector.memset(aug2[F:F + 1, :], 1.0)
            nc.vector.tensor_copy(out=aug2[:F, :], in_=tp2[:])
            ops = psB.tile([N_GRAPHS, N_CLASSES], f32, tag="g")
            nc.tensor.matmul(out=ops[:], lhsT=aug2[:], rhs=wl_sb[:],
                             start=True, stop=True)
            out_sb = wp.tile([N_GRAPHS, N_CLASSES], f32, tag="outsb")
            nc.vector.tensor_copy(out=out_sb[:], in_=ops[:])
            nc.sync.dma_start(out=out_d[:, :], in_=out_sb[:])

    nc.compile()
    return nc


def _run(inputs, trace=False):
    x = inputs["x"]
    prep = _host_prep(x, inputs["edge_index"], inputs["batch"], inputs["P_vec"])
    key = ("nc", prep["C"], tuple(prep["Kb"]))
    if key not in _cache:
        _cache.clear()
        _cache[key] = _build(prep["Kb"], prep["cbase"], prep["C"])
    nc = _cache[key]

    waugs = []
    for (W, b) in [(inputs["W1"], inputs["b1"]), (inputs["W2"], inputs["b2"]),
                   (inputs["W3"], inputs["b3"])]:
        waugs.append(np.concatenate(
            [np.asarray(W, np.float32), np.asarray(b, np.float32)[None, :]], axis=0))
    wlaug = np.concatenate(
        [np.asarray(inputs["Wl"], np.float32),
         np.asarray(inputs["bl"], np.float32)[None, :]], axis=0)
    iota64 = np.tile(np.arange(F, dtype=np.float32)[None, :], (128, 1))

    in_maps = []
    for c in range(NC):
        in_maps.append({
            "x_own": prep["x_slots"][c],
            "idx": prep["idx"][c], "pv": prep["P"][c],
            "gid": prep["gid"][c], "iota64": iota64,
            "waug0": waugs[0], "waug1": waugs[1], "waug2": waugs[2],
            "wlaug": wlaug,
        })

    res = bass_utils.run_bass_kernel_spmd(
        nc, in_maps, core_ids=list(range(NC)), trace=trace)
    return res.results[0]["out"].astype(np.float32), res


def kernel(**inputs) -> np.ndarray:
    out, _ = _run(inputs, trace=False)
    return out

